# revision 9
# baseline (speedup 1.0000x reference)
"""Bass/Trainium2 kernel for nn_DiscreteDecisionEngine (topk_masking).

Sharding: 1-way batch x 8-way class shard. Every core processes all 8192 rows;
core j owns classes [4000j, 4000j+4000) with its W.T shard resident in SBUF.
Per 128-row tile: fp32r matmuls into PSUM, fused bias*(1+eta) add on DVE,
exp + row-partial sums on ScalarE, 8-core AllReduce of softmax denominators,
candidate gather from the unnormalized exp row via ap_gather (16-call
rotation over the 16-partition groups), local argmax chain on DVE, then
normalize + write the probs shard. Host combines the 8 local argmax partials.

Self-contained: hardcodes all shapes for B=8192, D=512, C=32000, K=64.
"""

import os
import sys

import numpy as np

if "/opt/trn_rl_repo" not in sys.path:
    sys.path.insert(0, "/opt/trn_rl_repo")

B, D, C, K = 8192, 512, 32000, 64
N_CORES = 8
CS = C // N_CORES          # 4000 classes per core
RT = 64                    # row tiles
RP = 128                   # rows per tile (partitions)
CT = 8                     # c-tiles per core
CTW = CS // CT             # 500 classes per c-tile
KPAD = 32                  # padded candidate slots per (row, core)
BIGF = 3.0e38

N_TERMS = int(os.environ.get("KERNEL_N_TERMS", "1"))  # 1 = fp32r; 3 = split exact

_COMPILED = {}
LAST_EXEC_NS = None


def _round12(a):
    """Round to 12 explicit mantissa bits (conservative fp32r-representable)."""
    u = a.view(np.uint32) if a.dtype == np.float32 else a.astype(np.float32).view(np.uint32)
    # round-to-nearest on the low 11 bits
    u = (u.astype(np.uint64) + 0x400) & 0xFFFFF800
    return u.astype(np.uint32).view(np.float32)


def _build(n_terms):
    import concourse.bacc as bacc
    import concourse.mybir as mybir
    import concourse.tile as tile
    from concourse import library_config as lc

    F32 = mybir.dt.float32
    F32R = mybir.dt.float32r
    I16 = mybir.dt.int16
    I32 = mybir.dt.int32
    AF = mybir.ActivationFunctionType
    ALU = mybir.AluOpType
    AX = mybir.AxisListType

    nc = bacc.Bacc("TRN2", target_bir_lowering=False, debug=False)

    # ---- DRAM I/O ----
    F16 = mybir.dt.float16
    MMDT = F32R if n_terms == 1 else F16
    xt_d = nc.dram_tensor("xt", [D, B], MMDT, kind="ExternalInput")
    wt_d = nc.dram_tensor("wt", [D, CS], MMDT, kind="ExternalInput")
    if n_terms == 3:
        xtl_d = nc.dram_tensor("xtl", [D, B], F16, kind="ExternalInput")
        wtl_d = nc.dram_tensor("wtl", [D, CS], F16, kind="ExternalInput")
    biasr_d = nc.dram_tensor("biasr", [RP, CS], F32, kind="ExternalInput")
    etap_d = nc.dram_tensor("etap", [RP, RT], F32, kind="ExternalInput")
    idx_d = nc.dram_tensor("idx16", [RT, RP, 16, KPAD // 16], I16, kind="ExternalInput")
    candv_d = nc.dram_tensor("candv", [RT, RP, KPAD], F32, kind="ExternalInput")
    rotsel_d = nc.dram_tensor("rotsel", [RP, 16], F32, kind="ExternalInput")

    probs_d = nc.dram_tensor("probs", [B, CS], F32, kind="ExternalOutput")
    bestv_d = nc.dram_tensor("bestv", [RT, RP, 1], F32, kind="ExternalOutput")
    bestc_d = nc.dram_tensor("bestc", [RT, RP, 1], F32, kind="ExternalOutput")

    GB = 2  # row-tiles per collective batch
    NB = RT // GB
    ccb_in = nc.dram_tensor("ccb_in", [NB, RP, GB], F32)
    ccb_out = nc.dram_tensor("ccb_out", [NB, RP, GB], F32)

    with tile.TileContext(nc) as tc:
        with (
            tc.tile_pool(name="wpool", bufs=1) as wpool,
            tc.tile_pool(name="cpool", bufs=1) as cpool,
            tc.tile_pool(name="xpool", bufs=2) as xpool,
            tc.tile_pool(name="epool", bufs=5) as epool,
            tc.tile_pool(name="gpool", bufs=2) as gpool,
            tc.tile_pool(name="spool", bufs=3) as spool,
            tc.tile_pool(name="psum", bufs=4, space="PSUM") as pspool,
        ):
            nc.gpsimd.load_library(lc.ap_gather)

            # resident tensors
            wts = []
            for d in range(4):
                w_t = wpool.tile([RP, CS], MMDT, tag=f"w{d}")
                nc.sync.dma_start(w_t[:], wt_d[d * RP : (d + 1) * RP, :])
                wts.append(w_t)
            wtls = []
            if n_terms == 3:
                for d in range(4):
                    wl_t = wpool.tile([RP, CS], F16, tag=f"wl{d}")
                    nc.sync.dma_start(wl_t[:], wtl_d[d * RP : (d + 1) * RP, :])
                    wtls.append(wl_t)
            biasr = cpool.tile([RP, CS], F32, tag="biasr")
            nc.sync.dma_start(biasr[:], biasr_d[:])
            etap = cpool.tile([RP, RT], F32, tag="etap")
            nc.sync.dma_start(etap[:], etap_d[:])
            rotsel = cpool.tile([RP, 16], F32, tag="rotsel")
            nc.sync.dma_start(rotsel[:], rotsel_d[:])
            bigt = cpool.tile([RP, KPAD], F32, tag="bigt")
            nc.vector.memset(bigt[:], BIGF)

            rsum_pair = None
            pend = []  # (rt, exp_sb) awaiting normalize
            for rt in range(RT):
                xt_t = xpool.tile([RP, 4, RP], MMDT, tag="xt")
                nc.sync.dma_start(
                    xt_t[:],
                    xt_d.rearrange("(dt p) b -> p dt b", p=RP)[
                        :, :, rt * RP : (rt + 1) * RP
                    ],
                )
                if n_terms == 3:
                    xtl_t = xpool.tile([RP, 4, RP], F16, tag="xtl")
                    nc.sync.dma_start(
                        xtl_t[:],
                        xtl_d.rearrange("(dt p) b -> p dt b", p=RP)[
                            :, :, rt * RP : (rt + 1) * RP
                        ],
                    )
                idx_t = xpool.tile([RP, 16, KPAD // 16], I16, tag="idx")
                nc.sync.dma_start(idx_t[:], idx_d[rt])
                candv_t = xpool.tile([RP, KPAD], F32, tag="candv")
                nc.sync.dma_start(candv_t[:], candv_d[rt])

                exp_sb = epool.tile([RP, CS], F32, tag="exp")

                for ct in range(CT):
                    cs = slice(ct * CTW, (ct + 1) * CTW)
                    ps = pspool.tile([RP, CTW], F32, tag="ps")
                    if n_terms == 1:
                        mms = [(xt_t, wts)]
                    else:
                        mms = [(xt_t, wts), (xt_t, wtls), (xtl_t, wts)]
                    n_mm = len(mms) * 4
                    i = 0
                    for lt, rts_ in mms:
                        for d in range(4):
                            nc.tensor.matmul(
                                ps[:],
                                lt[:, d, :],
                                rts_[d][:, cs],
                                start=(i == 0),
                                stop=(i == n_mm - 1),
                            )
                            i += 1
                    # t = biasr*etap + psum  (DVE)
                    nc.vector.scalar_tensor_tensor(
                        out=exp_sb[:, cs],
                        in0=biasr[:, cs],
                        scalar=etap[:, rt : rt + 1],
                        in1=ps[:],
                        op0=ALU.mult,
                        op1=ALU.add,
                    )

                # one fused exp over the whole row, accumulating the row sum
                if rt % GB == 0:
                    rsum_pair = spool.tile([RP, GB], F32, tag="rsump")
                nc.scalar.activation(
                    exp_sb[:],
                    exp_sb[:],
                    AF.Exp,
                    accum_out=rsum_pair[:, rt % GB : rt % GB + 1],
                )

                # candidate gather + local argmax on unnormalized exp
                out_all = gpool.tile([RP, 16, KPAD], F32, tag="oa")
                for j in range(16):
                    nc.gpsimd.ap_gather(
                        out_all[:, j, :],
                        exp_sb[:],
                        idx_t[:, j, :],
                        channels=RP,
                        num_elems=CS,
                        d=1,
                        num_idxs=KPAD,
                    )
                nc.vector.tensor_mul(
                    out_all[:],
                    out_all[:],
                    rotsel[:, :, None].to_broadcast([RP, 16, KPAD]),
                )
                gath = spool.tile([RP, KPAD], F32, tag="gath")
                nc.vector.reduce_sum(
                    gath[:], out_all[:].rearrange("p j k -> p k j"), axis=AX.X
                )
                bv = spool.tile([RP, 1], F32, tag="bv")
                nc.vector.reduce_max(bv[:], gath[:], axis=AX.X)
                eqm = spool.tile([RP, KPAD], I32, tag="eqm")
                nc.vector.tensor_scalar(eqm[:], gath[:], bv[:], None, op0=ALU.is_equal)
                selv = spool.tile([RP, KPAD], F32, tag="selv")
                nc.vector.select(selv[:], eqm[:], candv_t[:], bigt[:])
                bc = spool.tile([RP, 1], F32, tag="bc")
                nc.vector.tensor_reduce(bc[:], selv[:], axis=AX.X, op=ALU.min)
                nc.sync.dma_start(bestv_d[rt], bv[:])
                nc.sync.dma_start(bestc_d[rt], bc[:])

                pend.append((rt, exp_sb))

                # one 8-core AllReduce per GB row-tiles, then normalize the batch
                if rt % GB == GB - 1:
                    pr = rt // GB
                    nc.sync.dma_start(ccb_in[pr], rsum_pair[:])
                    nc.gpsimd.collective_compute(
                        "AllReduce",
                        ALU.add,
                        replica_groups=[list(range(N_CORES))],
                        ins=[ccb_in[pr]],
                        outs=[ccb_out[pr]],
                    )
                    rsg = spool.tile([RP, GB], F32, tag="rsg")
                    nc.sync.dma_start(rsg[:], ccb_out[pr])
                    rcp2 = spool.tile([RP, GB], F32, tag="rcp")
                    nc.vector.reciprocal(rcp2[:], rsg[:])
                    for r2, e2 in pend:
                        nc.scalar.activation(
                            e2[:], e2[:], AF.Copy, bias=0.0,
                            scale=rcp2[:, r2 % GB : r2 % GB + 1],
                        )
                        nc.sync.dma_start(probs_d[r2 * RP : (r2 + 1) * RP, :], e2[:])
                    pend = []

    nc.compile()
    return nc


def _host_prep(x, candidate_actions, eta, W, b, n_terms):
    x = np.asarray(x, dtype=np.float32)
    cand = np.asarray(candidate_actions)
    eta = np.asarray(eta, dtype=np.float32)
    W = np.asarray(W, dtype=np.float32)
    b = np.asarray(b, dtype=np.float32)

    scale = 1.0 + eta  # [B]
    xs = (x * scale[:, None]).astype(np.float32)  # scaled rows
    if n_terms == 3:
        xh = xs.astype(np.float16)
        xl = (xs - xh.astype(np.float32)).astype(np.float16)
        Wh = W.astype(np.float16)
        Wl = (W - Wh.astype(np.float32)).astype(np.float16)
        xt = np.ascontiguousarray(xh.T)
        xtl = np.ascontiguousarray(xl.T)
    else:
        xt = np.ascontiguousarray(xs.T)  # [D, B]
        xtl = None
        Wh, Wl = W, None

    WT = np.ascontiguousarray(Wh.T)  # [D, C]
    WTl = np.ascontiguousarray(Wl.T) if n_terms == 3 else None

    etap_all = np.ascontiguousarray(scale.reshape(RT, RP).T)  # [RP, RT]
    rotsel = (
        (np.arange(RP) % 16)[:, None] == np.arange(16)[None, :]
    ).astype(np.float32)

    cand_i = cand.astype(np.int64)
    owner = cand_i // CS  # [B, K]
    in_maps = []
    empty_masks = []
    for j in range(N_CORES):
        mask = owner == j  # [B, K]
        counts = mask.sum(axis=1)
        assert counts.max() <= KPAD, f"KPAD too small: {counts.max()}"
        order = np.argsort(~mask, axis=1, kind="stable")[:, :KPAD]  # valid first
        sel_valid = np.take_along_axis(mask, order, axis=1)  # [B, KPAD]
        gsel = np.take_along_axis(cand_i, order, axis=1)  # global classes
        loc = (gsel - j * CS).astype(np.int64)
        # pad: duplicate the first valid local index (or 0 when the row is empty)
        first_loc = np.where(counts > 0, loc[:, 0], 0)
        loc = np.where(sel_valid, loc, first_loc[:, None]).astype(np.int16)
        cv = np.where(sel_valid, gsel.astype(np.float32), BIGF).astype(np.float32)

        # wrapped gather-index layout: idx16[rt, 16g+pp, jj, s] = loc[row(rt,g,jj), 16s+pp]
        L = loc.reshape(RT, 8, 16, KPAD // 16, 16)  # [rt, g, jj, s, pp]
        idx16 = np.ascontiguousarray(L.transpose(0, 1, 4, 2, 3)).reshape(
            RT, RP, 16, KPAD // 16
        )
        candv = np.ascontiguousarray(cv.reshape(RT, RP, KPAD))

        m = {
            "xt": xt,
            "wt": np.ascontiguousarray(WT[:, j * CS : (j + 1) * CS]),
            "biasr": np.ascontiguousarray(
                np.broadcast_to(b[j * CS : (j + 1) * CS], (RP, CS))
            ),
            "etap": etap_all,
            "idx16": idx16,
            "candv": candv,
            "rotsel": rotsel,
        }
        if n_terms == 3:
            m["xtl"] = xtl
            m["wtl"] = np.ascontiguousarray(WTl[:, j * CS : (j + 1) * CS])
        in_maps.append(m)
        empty_masks.append(counts == 0)
    return in_maps, empty_masks


def kernel(x, candidate_actions, eta, W, b):
    global LAST_EXEC_NS
    from concourse import bass_utils

    bass_utils.upload_artifacts = lambda tmpdir: f"local://{tmpdir}"

    n_terms = N_TERMS
    if n_terms not in _COMPILED:
        _COMPILED[n_terms] = _build(n_terms)
    nc = _COMPILED[n_terms]

    in_maps, empty_masks = _host_prep(x, candidate_actions, eta, W, b, n_terms)

    trace = os.environ.get("BASS_KERNEL_TRACE", "0") == "1"
    res = bass_utils.run_bass_kernel_spmd(
        nc, in_maps, list(range(N_CORES)), trace=trace
    )
    LAST_EXEC_NS = res.exec_time_ns
    globals()["LAST_RESULT"] = res

    # assemble probs [B, C]
    probs = np.empty((B, C), dtype=np.float32)
    for j in range(N_CORES):
        probs[:, j * CS : (j + 1) * CS] = res.results[j]["probs"]

    # combine argmax partials
    bestv = np.stack(
        [res.results[j]["bestv"].reshape(B) for j in range(N_CORES)], axis=0
    )  # [8, B]
    bestc = np.stack(
        [res.results[j]["bestc"].reshape(B) for j in range(N_CORES)], axis=0
    )
    for j in range(N_CORES):
        bestv[j][empty_masks[j]] = -1.0  # empty shards never win
    winner = np.argmax(bestv, axis=0)
    chosen_f = bestc[winner, np.arange(B)]
    chosen = chosen_f.astype(np.asarray(candidate_actions).dtype)

    eta_values = np.full((B,), 0.5, dtype=np.float32)
    return chosen, eta_values, probs
